# revision 1
# baseline (speedup 1.0000x reference)
"""Dirichlet energy loss (ball-query KNN graph) on 8 Trainium2 cores.

For each point i in a cloud of N=4096 points: find its (up to) K=32 nearest
neighbors within radius R=0.15, sum (f_i - f_j)^2 over them, then return
0.5 * mean over all points/batches.

Strategy (data-parallel over B=8, one cloud per NeuronCore):
  host:   sort each cloud by x; neighbors of a 128-row tile then lie in a
          narrow rank window of columns (verified exactly via searchsorted,
          fallback = full width). Precompute matmul operands so the device
          computes u_ij = r^2 - d^2_ij with one tiny-K matmul + one ACT op.
  device: per row tile: PE matmul (K=4 contraction) -> u in PSUM;
          ACT adds per-row bias -> u0; 4x vector.max (top-8) + 3x
          match_replace extract the 32nd-largest u (= distance threshold,
          clamped at 0 == radius); one fused scalar_tensor_tensor computes
          sum_j (u0 >= t) * (f_i - f_j)^2 per row (G = (f_i-f_j)^2 comes
          from one ACT Square with per-partition bias).
  host:   sum the per-row partials from all cores, multiply by 0.5/(B*N).
"""

import numpy as np

R = 0.15
RSQ = R * R
K = 32
B = 8
N = 4096
NTILES = N // 128
MARGIN_FAST = 768  # rank-window half-width; actual data needs <= 640
BIG_NEG = -3.0e38

_kernel_cache = {}


def _build_bass(margin):
    import concourse.bacc as bacc
    import concourse.tile as tile
    from concourse import mybir

    f32 = mybir.dt.float32
    windows = [
        (max(0, 128 * t - margin), min(N, 128 * (t + 1) + margin))
        for t in range(NTILES)
    ]
    wmax = max(ce - cs for cs, ce in windows)
    psum_w = ((wmax + 511) // 512) * 512

    nc = bacc.Bacc("TRN2", target_bir_lowering=False, debug=False, num_devices=B)
    lhsT_d = nc.dram_tensor("lhsT", [4, N], f32, kind="ExternalInput")
    rhs_d = nc.dram_tensor("rhs", [4, N], f32, kind="ExternalInput")
    f_d = nc.dram_tensor("fvals", [1, N], f32, kind="ExternalInput")
    bias_d = nc.dram_tensor("biascol", [128, NTILES], f32, kind="ExternalInput")
    nf_d = nc.dram_tensor("nfcol", [128, NTILES], f32, kind="ExternalInput")
    out_d = nc.dram_tensor("partials", [128, NTILES], f32, kind="ExternalOutput")

    with tile.TileContext(nc) as tc:
        with (
            tc.tile_pool(name="const", bufs=1) as cpool,
            tc.tile_pool(name="work", bufs=2) as wpool,
            tc.tile_pool(name="small", bufs=3) as spool,
            tc.tile_pool(name="psum", bufs=2, space="PSUM") as ppool,
        ):
            lhsT_sb = cpool.tile([4, N], f32, tag="lhsT")
            rhs_sb = cpool.tile([4, N], f32, tag="rhs")
            f_row = cpool.tile([1, N], f32, tag="frow")
            F = cpool.tile([128, N], f32, tag="F")
            bias_sb = cpool.tile([128, NTILES], f32, tag="bias")
            nf_sb = cpool.tile([128, NTILES], f32, tag="nf")
            partials = cpool.tile([128, NTILES], f32, tag="partials")

            nc.sync.dma_start(lhsT_sb[:], lhsT_d.ap()[:])
            nc.sync.dma_start(rhs_sb[:], rhs_d.ap()[:])
            nc.sync.dma_start(f_row[:], f_d.ap()[:])
            nc.sync.dma_start(bias_sb[:], bias_d.ap()[:])
            nc.sync.dma_start(nf_sb[:], nf_d.ap()[:])
            nc.gpsimd.partition_broadcast(F[:], f_row[:])

            for t in range(NTILES):
                cs, ce = windows[t]
                w = ce - cs
                ps = ppool.tile([128, psum_w], f32, tag="ps")
                for off in range(0, w, 512):
                    cw = min(512, w - off)
                    nc.tensor.matmul(
                        ps[:, off : off + cw],
                        lhsT_sb[:, 128 * t : 128 * (t + 1)],
                        rhs_sb[:, cs + off : cs + off + cw],
                        start=True,
                        stop=True,
                    )
                u0 = wpool.tile([128, wmax], f32, tag="u0")
                nc.scalar.activation(
                    u0[:, :w],
                    ps[:, :w],
                    mybir.ActivationFunctionType.Identity,
                    bias=bias_sb[:, t : t + 1],
                )
                G = wpool.tile([128, wmax], f32, tag="G")
                nc.scalar.activation(
                    G[:, :w],
                    F[:, cs:ce],
                    mybir.ActivationFunctionType.Square,
                    bias=nf_sb[:, t : t + 1],
                )
                m8a = spool.tile([128, 8], f32, tag="m8a")
                m8b = spool.tile([128, 8], f32, tag="m8b")
                m8c = spool.tile([128, 8], f32, tag="m8c")
                m8d = spool.tile([128, 8], f32, tag="m8d")
                v1 = wpool.tile([128, wmax], f32, tag="v1")
                v2 = wpool.tile([128, wmax], f32, tag="v2")
                v3 = wpool.tile([128, wmax], f32, tag="v3")
                nc.vector.max(out=m8a[:], in_=u0[:, :w])
                nc.vector.match_replace(
                    out=v1[:, :w], in_to_replace=m8a[:], in_values=u0[:, :w],
                    imm_value=BIG_NEG,
                )
                nc.vector.max(out=m8b[:], in_=v1[:, :w])
                nc.vector.match_replace(
                    out=v2[:, :w], in_to_replace=m8b[:], in_values=v1[:, :w],
                    imm_value=BIG_NEG,
                )
                nc.vector.max(out=m8c[:], in_=v2[:, :w])
                nc.vector.match_replace(
                    out=v3[:, :w], in_to_replace=m8c[:], in_values=v2[:, :w],
                    imm_value=BIG_NEG,
                )
                nc.vector.max(out=m8d[:], in_=v3[:, :w])
                teff = spool.tile([128, 1], f32, tag="teff")
                nc.scalar.activation(
                    teff[:], m8d[:, 7:8], mybir.ActivationFunctionType.Relu
                )
                scratch = wpool.tile([128, wmax], f32, tag="scratch")
                nc.vector.scalar_tensor_tensor(
                    out=scratch[:, :w],
                    in0=u0[:, :w],
                    scalar=teff[:],
                    in1=G[:, :w],
                    op0=mybir.AluOpType.is_ge,
                    op1=mybir.AluOpType.mult,
                    accum_out=partials[:, t : t + 1],
                )
            nc.sync.dma_start(out_d.ap()[:], partials[:])

    nc.compile()
    return nc


def _get_kernel(margin):
    if margin not in _kernel_cache:
        _kernel_cache[margin] = _build_bass(margin)
    return _kernel_cache[margin]


def _prep_core(pos_b, f_b):
    """Host-side preprocessing for one cloud -> input map + needed margin."""
    order = np.argsort(pos_b[:, 0], kind="stable")
    p = pos_b[order].astype(np.float32)
    fs = f_b[order].astype(np.float32)
    c = (p.astype(np.float64) - 0.5)
    n = (c * c).sum(-1)
    c32 = c.astype(np.float32)

    lhsT = np.empty((4, N), np.float32)
    lhsT[0:3] = c32.T
    lhsT[3] = 1.0
    rhs = np.empty((4, N), np.float32)
    rhs[0:3] = 2.0 * c32.T
    rhs[3] = (-n).astype(np.float32)
    biascol = np.ascontiguousarray(
        (RSQ - n).astype(np.float32).reshape(NTILES, 128).T
    )
    nfcol = np.ascontiguousarray((-fs).reshape(NTILES, 128).T)
    fvals = fs.reshape(1, N)

    # exact per-tile window requirement (rank space)
    xs = p[:, 0].astype(np.float64)
    starts = xs[0::128]
    ends = xs[127::128]
    lo = np.searchsorted(xs, starts - R, side="left")
    hi = np.searchsorted(xs, ends + R, side="right")
    t_idx = np.arange(NTILES)
    need = max(
        int((128 * t_idx - lo).max()),
        int((hi - 128 * (t_idx + 1)).max()),
    )
    in_map = {
        "lhsT": lhsT,
        "rhs": rhs,
        "fvals": fvals,
        "biascol": biascol,
        "nfcol": nfcol,
    }
    return in_map, need


def prepare_inputs(pos, f):
    """Returns (in_maps, margin) for the 8 cores."""
    pos = np.asarray(pos, dtype=np.float32)
    f = np.asarray(f, dtype=np.float32)
    assert pos.shape == (B, N, 3), pos.shape
    assert f.shape == (B, N), f.shape
    in_maps, needs = [], []
    for b in range(B):
        m, need = _prep_core(pos[b], f[b])
        in_maps.append(m)
        needs.append(need)
    margin = MARGIN_FAST if max(needs) <= MARGIN_FAST else N
    return in_maps, margin


def finish(results):
    total = 0.0
    for rmap in results:
        total += rmap["partials"].astype(np.float64).sum()
    return np.asarray(0.5 * total / (B * N), dtype=np.float32)


def kernel(pos, f):
    from concourse.bass_utils import run_bass_kernel_spmd

    in_maps, margin = prepare_inputs(pos, f)
    nc = _get_kernel(margin)
    res = run_bass_kernel_spmd(nc, in_maps, list(range(B)))
    return finish(res.results)


# revision 5
# speedup vs baseline: 224.3191x; 224.3191x over previous
"""Dirichlet energy loss (ball-query KNN graph) on 8 Trainium2 cores.

For each point i in a cloud of N=4096 points: find its (up to) K=32 nearest
neighbors within radius R=0.15, sum (f_i - f_j)^2 over them, then return
0.5 * mean over all points/batches.

Strategy (data-parallel over B=8, one cloud per NeuronCore):
  host:   sort each cloud by x; neighbors of a 128-row tile then lie in a
          narrow rank window of columns (verified exactly via searchsorted,
          fallback = full width). Precompute matmul operands so the device
          computes u_ij = r^2 - d^2_ij with one tiny-K matmul + one ACT op.
  device: per row tile: PE matmul (K=4 contraction) -> u in PSUM;
          ACT adds per-row bias -> u0; 4x vector.max (top-8) + 3x
          match_replace extract the 32nd-largest u (= distance threshold,
          clamped at 0 == radius); one fused scalar_tensor_tensor computes
          sum_j (u0 >= t) * (f_i - f_j)^2 per row (G = (f_i-f_j)^2 comes
          from one ACT Square with per-partition bias).
  host:   sum the per-row partials from all cores, multiply by 0.5/(B*N).
"""

import numpy as np

R = 0.15
RSQ = R * R
K = 32
B = 8
N = 4096
NTILES = N // 128
MARGIN_FAST = 768  # rank-window half-width; actual data needs <= 640
BIG_NEG = -3.0e38

_kernel_cache = {}


def _build_bass(margin, rep=1):
    import contextlib
    import concourse.bacc as bacc
    import concourse.tile as tile
    from concourse import mybir

    f32 = mybir.dt.float32
    windows = [
        (max(0, 128 * t - margin), min(N, 128 * (t + 1) + margin))
        for t in range(NTILES)
    ]
    wmax = max(ce - cs for cs, ce in windows)
    psum_w = ((wmax + 511) // 512) * 512

    nc = bacc.Bacc("TRN2", target_bir_lowering=False, debug=False, num_devices=B)
    lhsT_d = nc.dram_tensor("lhsT", [4, N], f32, kind="ExternalInput")
    rhs_d = nc.dram_tensor("rhs", [4, N], f32, kind="ExternalInput")
    f_d = nc.dram_tensor("fvals", [1, N], f32, kind="ExternalInput")
    bias_d = nc.dram_tensor("biascol", [128, NTILES], f32, kind="ExternalInput")
    nf_d = nc.dram_tensor("nfcol", [128, NTILES], f32, kind="ExternalInput")
    out_d = nc.dram_tensor("partials", [128, NTILES], f32, kind="ExternalOutput")

    with tile.TileContext(nc) as tc:
        with (
            tc.tile_pool(name="const", bufs=1) as cpool,
            tc.tile_pool(name="work", bufs=2) as wpool,
            tc.tile_pool(name="small", bufs=3) as spool,
            tc.tile_pool(name="psum", bufs=2, space="PSUM") as ppool,
        ):
            lhsT_sb = cpool.tile([4, N], f32, tag="lhsT")
            rhs_sb = cpool.tile([4, N], f32, tag="rhs")
            f_row = cpool.tile([1, N], f32, tag="frow")
            F = cpool.tile([128, N], f32, tag="F")
            bias_sb = cpool.tile([128, NTILES], f32, tag="bias")
            nf_sb = cpool.tile([128, NTILES], f32, tag="nf")
            partials = cpool.tile([128, NTILES], f32, tag="partials")

            nc.sync.dma_start(lhsT_sb[:], lhsT_d.ap()[:])
            nc.sync.dma_start(rhs_sb[:], rhs_d.ap()[:])
            nc.sync.dma_start(f_row[:], f_d.ap()[:])
            nc.sync.dma_start(bias_sb[:], bias_d.ap()[:])
            nc.sync.dma_start(nf_sb[:], nf_d.ap()[:])
            nc.gpsimd.partition_broadcast(F[:], f_row[:])

            rep_ctx = tc.For_i(0, rep, 1) if rep > 1 else contextlib.nullcontext()
            with rep_ctx:
                _emit_tiles(nc, tc, mybir, windows, wmax, psum_w, wpool, spool,
                            ppool, lhsT_sb, rhs_sb, F, bias_sb, nf_sb, partials)
            nc.sync.dma_start(out_d.ap()[:], partials[:])

    nc.compile()
    return nc


def _emit_tiles(nc, tc, mybir, windows, wmax, psum_w, wpool, spool, ppool,
                lhsT_sb, rhs_sb, F, bias_sb, nf_sb, partials):
    f32 = mybir.dt.float32
    for t in range(NTILES):
        cs, ce = windows[t]
        w = ce - cs
        ps = ppool.tile([128, psum_w], f32, tag="ps")
        for off in range(0, w, 512):
            cw = min(512, w - off)
            nc.tensor.matmul(
                ps[:, off : off + cw],
                lhsT_sb[:, 128 * t : 128 * (t + 1)],
                rhs_sb[:, cs + off : cs + off + cw],
                start=True,
                stop=True,
            )
        u0 = wpool.tile([128, wmax], f32, tag="u0")
        nc.scalar.activation(
            u0[:, :w],
            ps[:, :w],
            mybir.ActivationFunctionType.Identity,
            bias=bias_sb[:, t : t + 1],
        )
        G = wpool.tile([128, wmax], f32, tag="G")
        nc.scalar.activation(
            G[:, :w],
            F[:, cs:ce],
            mybir.ActivationFunctionType.Square,
            bias=nf_sb[:, t : t + 1],
        )
        m8a = spool.tile([128, 8], f32, tag="m8a")
        m8b = spool.tile([128, 8], f32, tag="m8b")
        m8c = spool.tile([128, 8], f32, tag="m8c")
        m8d = spool.tile([128, 8], f32, tag="m8d")
        v1 = wpool.tile([128, wmax], f32, tag="v1")
        v2 = wpool.tile([128, wmax], f32, tag="v2")
        v3 = wpool.tile([128, wmax], f32, tag="v3")
        nc.vector.max(out=m8a[:], in_=u0[:, :w])
        nc.vector.match_replace(
            out=v1[:, :w], in_to_replace=m8a[:], in_values=u0[:, :w],
            imm_value=BIG_NEG,
        )
        nc.vector.max(out=m8b[:], in_=v1[:, :w])
        nc.vector.match_replace(
            out=v2[:, :w], in_to_replace=m8b[:], in_values=v1[:, :w],
            imm_value=BIG_NEG,
        )
        nc.vector.max(out=m8c[:], in_=v2[:, :w])
        nc.vector.match_replace(
            out=v3[:, :w], in_to_replace=m8c[:], in_values=v2[:, :w],
            imm_value=BIG_NEG,
        )
        nc.vector.max(out=m8d[:], in_=v3[:, :w])
        teff = spool.tile([128, 1], f32, tag="teff")
        nc.scalar.activation(
            teff[:], m8d[:, 7:8], mybir.ActivationFunctionType.Relu
        )
        scratch = wpool.tile([128, wmax], f32, tag="scratch")
        nc.vector.scalar_tensor_tensor(
            out=scratch[:, :w],
            in0=u0[:, :w],
            scalar=teff[:],
            in1=G[:, :w],
            op0=mybir.AluOpType.is_ge,
            op1=mybir.AluOpType.mult,
            accum_out=partials[:, t : t + 1],
        )


def _get_kernel(margin):
    if margin not in _kernel_cache:
        _kernel_cache[margin] = _build_bass(margin)
    return _kernel_cache[margin]


def _prep_core(pos_b, f_b):
    """Host-side preprocessing for one cloud -> input map + needed margin."""
    order = np.argsort(pos_b[:, 0], kind="stable")
    p = pos_b[order].astype(np.float32)
    fs = f_b[order].astype(np.float32)
    c = (p.astype(np.float64) - 0.5)
    n = (c * c).sum(-1)
    c32 = c.astype(np.float32)

    lhsT = np.empty((4, N), np.float32)
    lhsT[0:3] = c32.T
    lhsT[3] = 1.0
    rhs = np.empty((4, N), np.float32)
    rhs[0:3] = 2.0 * c32.T
    rhs[3] = (-n).astype(np.float32)
    biascol = np.ascontiguousarray(
        (RSQ - n).astype(np.float32).reshape(NTILES, 128).T
    )
    nfcol = np.ascontiguousarray((-fs).reshape(NTILES, 128).T)
    fvals = fs.reshape(1, N)

    # exact per-tile window requirement (rank space)
    xs = p[:, 0].astype(np.float64)
    starts = xs[0::128]
    ends = xs[127::128]
    lo = np.searchsorted(xs, starts - R, side="left")
    hi = np.searchsorted(xs, ends + R, side="right")
    t_idx = np.arange(NTILES)
    need = max(
        int((128 * t_idx - lo).max()),
        int((hi - 128 * (t_idx + 1)).max()),
    )
    in_map = {
        "lhsT": lhsT,
        "rhs": rhs,
        "fvals": fvals,
        "biascol": biascol,
        "nfcol": nfcol,
    }
    return in_map, need


def prepare_inputs(pos, f):
    """Returns (in_maps, margin) for the 8 cores."""
    pos = np.asarray(pos, dtype=np.float32)
    f = np.asarray(f, dtype=np.float32)
    assert pos.shape == (B, N, 3), pos.shape
    assert f.shape == (B, N), f.shape
    in_maps, needs = [], []
    for b in range(B):
        m, need = _prep_core(pos[b], f[b])
        in_maps.append(m)
        needs.append(need)
    margin = MARGIN_FAST if max(needs) <= MARGIN_FAST else N
    return in_maps, margin


def finish(results):
    total = 0.0
    for rmap in results:
        total += rmap["partials"].astype(np.float64).sum()
    return np.asarray(0.5 * total / (B * N), dtype=np.float32)


def kernel(pos, f):
    from concourse.bass_utils import run_bass_kernel_spmd

    in_maps, margin = prepare_inputs(pos, f)
    nc = _get_kernel(margin)
    res = run_bass_kernel_spmd(nc, in_maps, list(range(B)))
    return finish(res.results)


# revision 7
# speedup vs baseline: 459.0543x; 2.0464x over previous
"""Dirichlet energy loss (ball-query KNN graph) on 8 Trainium2 cores.

For each point i in a cloud of N=4096 points: find its (up to) K=32 nearest
neighbors within radius R=0.15, sum (f_i - f_j)^2 over them, then return
0.5 * mean over all points/batches.

Strategy (data-parallel over B=8, one cloud per NeuronCore):
  host:   sort each cloud by x; neighbors of a 128-row tile then lie in a
          narrow rank window of columns (verified exactly via searchsorted,
          fallback = full width). Precompute matmul operands so the device
          computes u_ij = r^2 - d^2_ij with one tiny-K matmul + one ACT op.
  device: per row tile: PE matmul (K=4 contraction) -> u in PSUM;
          ACT adds per-row bias -> u0; 4x vector.max (top-8) + 3x
          match_replace extract the 32nd-largest u (= distance threshold,
          clamped at 0 == radius); one fused scalar_tensor_tensor computes
          sum_j (u0 >= t) * (f_i - f_j)^2 per row (G = (f_i-f_j)^2 comes
          from one ACT Square with per-partition bias).
  host:   sum the per-row partials from all cores, multiply by 0.5/(B*N).
"""

import numpy as np

R = 0.15
RSQ = R * R
K = 32
B = 8
N = 4096
NTILES = N // 128
MARGIN_FAST = 704  # rank-window half-width; actual data needs <= 640
BIG_NEG = -3.0e38

_kernel_cache = {}


def _build_bass(margin, rep=1):
    import contextlib
    import concourse.bacc as bacc
    import concourse.tile as tile
    from concourse import mybir

    f32 = mybir.dt.float32
    windows = [
        (max(0, 128 * t - margin), min(N, 128 * (t + 1) + margin))
        for t in range(NTILES)
    ]
    wmax = max(ce - cs for cs, ce in windows)
    psum_w = ((wmax + 511) // 512) * 512

    nc = bacc.Bacc("TRN2", target_bir_lowering=False, debug=False, num_devices=B)
    lhsT_d = nc.dram_tensor("lhsT", [4, N], f32, kind="ExternalInput")
    rhs_d = nc.dram_tensor("rhs", [4, N], f32, kind="ExternalInput")
    f_d = nc.dram_tensor("fvals", [1, N], f32, kind="ExternalInput")
    bias_d = nc.dram_tensor("biascol", [128, NTILES], f32, kind="ExternalInput")
    nf_d = nc.dram_tensor("nfcol", [128, NTILES], f32, kind="ExternalInput")
    out_d = nc.dram_tensor("partials", [128, NTILES], f32, kind="ExternalOutput")

    with tile.TileContext(nc) as tc:
        with (
            tc.tile_pool(name="const", bufs=1) as cpool,
            tc.tile_pool(name="work", bufs=2) as wpool,
            tc.tile_pool(name="small", bufs=3) as spool,
            tc.tile_pool(name="psum", bufs=2, space="PSUM") as ppool,
        ):
            lhsT_sb = cpool.tile([4, N], f32, tag="lhsT")
            rhs_sb = cpool.tile([4, N], f32, tag="rhs")
            f_row = cpool.tile([1, N], f32, tag="frow")
            F = cpool.tile([128, N], f32, tag="F")
            bias_sb = cpool.tile([128, NTILES], f32, tag="bias")
            nf_sb = cpool.tile([128, NTILES], f32, tag="nf")
            partials = cpool.tile([128, NTILES], f32, tag="partials")

            nc.sync.dma_start(lhsT_sb[:], lhsT_d.ap()[:])
            nc.sync.dma_start(rhs_sb[:], rhs_d.ap()[:])
            nc.sync.dma_start(f_row[:], f_d.ap()[:])
            nc.sync.dma_start(bias_sb[:], bias_d.ap()[:])
            nc.sync.dma_start(nf_sb[:], nf_d.ap()[:])
            nc.gpsimd.partition_broadcast(F[:], f_row[:])

            rep_ctx = tc.For_i(0, rep, 1) if rep > 1 else contextlib.nullcontext()
            with rep_ctx:
                _emit_tiles(nc, tc, mybir, windows, wmax, psum_w, wpool, spool,
                            ppool, lhsT_sb, rhs_sb, F, bias_sb, nf_sb, partials)
            nc.sync.dma_start(out_d.ap()[:], partials[:])

    nc.compile()
    return nc


def _emit_tiles(nc, tc, mybir, windows, wmax, psum_w, wpool, spool, ppool,
                lhsT_sb, rhs_sb, F, bias_sb, nf_sb, partials):
    f32 = mybir.dt.float32
    NG = 16  # interleaved candidate groups per row
    for t in range(NTILES):
        cs, ce = windows[t]
        w = ce - cs
        assert w % NG == 0, (t, w)
        wg = w // NG
        ps = ppool.tile([128, psum_w], f32, tag="ps")
        for off in range(0, w, 512):
            cw = min(512, w - off)
            nc.tensor.matmul(
                ps[:, off : off + cw],
                lhsT_sb[:, 128 * t : 128 * (t + 1)],
                rhs_sb[:, cs + off : cs + off + cw],
                start=True,
                stop=True,
            )
        # u0/G are written in "grouped" layout: element j of the window lands
        # at [g*wg + k] where j = k*NG + g. Group g is then a contiguous
        # slice holding every NG-th candidate -> the top-32 of a row spreads
        # ~uniformly over groups, so the union of per-group top-8s contains
        # the true top-32 (up to ~1e-4 probability per row).
        u0 = wpool.tile([128, wmax], f32, tag="u0")
        out_ap = u0[:, :w].rearrange("p (g k) -> p k g", g=NG)
        in_ap = ps[:, :w].rearrange("p (k g) -> p k g", g=NG)
        nc.scalar.activation(
            out_ap,
            in_ap,
            mybir.ActivationFunctionType.Identity,
            bias=bias_sb[:, t : t + 1],
        )
        G = wpool.tile([128, wmax], f32, tag="G")
        nc.scalar.activation(
            G[:, :w].rearrange("p (g k) -> p k g", g=NG),
            F[:, cs:ce].rearrange("p (k g) -> p k g", g=NG),
            mybir.ActivationFunctionType.Square,
            bias=nf_sb[:, t : t + 1],
        )
        cand = spool.tile([128, 8 * NG], f32, tag="cand")
        for g in range(NG):
            nc.vector.max(
                out=cand[:, 8 * g : 8 * g + 8], in_=u0[:, g * wg : (g + 1) * wg]
            )
        m8a = spool.tile([128, 8], f32, tag="m8a")
        m8b = spool.tile([128, 8], f32, tag="m8b")
        m8c = spool.tile([128, 8], f32, tag="m8c")
        m8d = spool.tile([128, 8], f32, tag="m8d")
        v1 = spool.tile([128, 8 * NG], f32, tag="v1")
        v2 = spool.tile([128, 8 * NG], f32, tag="v2")
        v3 = spool.tile([128, 8 * NG], f32, tag="v3")
        nc.vector.max(out=m8a[:], in_=cand[:])
        nc.vector.match_replace(
            out=v1[:], in_to_replace=m8a[:], in_values=cand[:], imm_value=BIG_NEG
        )
        nc.vector.max(out=m8b[:], in_=v1[:])
        nc.vector.match_replace(
            out=v2[:], in_to_replace=m8b[:], in_values=v1[:], imm_value=BIG_NEG
        )
        nc.vector.max(out=m8c[:], in_=v2[:])
        nc.vector.match_replace(
            out=v3[:], in_to_replace=m8c[:], in_values=v2[:], imm_value=BIG_NEG
        )
        nc.vector.max(out=m8d[:], in_=v3[:])
        teff = spool.tile([128, 1], f32, tag="teff")
        nc.scalar.activation(
            teff[:], m8d[:, 7:8], mybir.ActivationFunctionType.Relu
        )
        scratch = wpool.tile([128, wmax], f32, tag="scratch")
        nc.vector.scalar_tensor_tensor(
            out=scratch[:, :w],
            in0=u0[:, :w],
            scalar=teff[:],
            in1=G[:, :w],
            op0=mybir.AluOpType.is_ge,
            op1=mybir.AluOpType.mult,
            accum_out=partials[:, t : t + 1],
        )


def _get_kernel(margin):
    if margin not in _kernel_cache:
        _kernel_cache[margin] = _build_bass(margin)
    return _kernel_cache[margin]


def _prep_core(pos_b, f_b):
    """Host-side preprocessing for one cloud -> input map + needed margin."""
    order = np.argsort(pos_b[:, 0], kind="stable")
    p = pos_b[order].astype(np.float32)
    fs = f_b[order].astype(np.float32)
    c = (p.astype(np.float64) - 0.5)
    n = (c * c).sum(-1)
    c32 = c.astype(np.float32)

    lhsT = np.empty((4, N), np.float32)
    lhsT[0:3] = c32.T
    lhsT[3] = 1.0
    rhs = np.empty((4, N), np.float32)
    rhs[0:3] = 2.0 * c32.T
    rhs[3] = (-n).astype(np.float32)
    biascol = np.ascontiguousarray(
        (RSQ - n).astype(np.float32).reshape(NTILES, 128).T
    )
    nfcol = np.ascontiguousarray((-fs).reshape(NTILES, 128).T)
    fvals = fs.reshape(1, N)

    # exact per-tile window requirement (rank space)
    xs = p[:, 0].astype(np.float64)
    starts = xs[0::128]
    ends = xs[127::128]
    lo = np.searchsorted(xs, starts - R, side="left")
    hi = np.searchsorted(xs, ends + R, side="right")
    t_idx = np.arange(NTILES)
    need = max(
        int((128 * t_idx - lo).max()),
        int((hi - 128 * (t_idx + 1)).max()),
    )
    in_map = {
        "lhsT": lhsT,
        "rhs": rhs,
        "fvals": fvals,
        "biascol": biascol,
        "nfcol": nfcol,
    }
    return in_map, need


def prepare_inputs(pos, f):
    """Returns (in_maps, margin) for the 8 cores."""
    pos = np.asarray(pos, dtype=np.float32)
    f = np.asarray(f, dtype=np.float32)
    assert pos.shape == (B, N, 3), pos.shape
    assert f.shape == (B, N), f.shape
    in_maps, needs = [], []
    for b in range(B):
        m, need = _prep_core(pos[b], f[b])
        in_maps.append(m)
        needs.append(need)
    margin = MARGIN_FAST if max(needs) <= MARGIN_FAST else N
    return in_maps, margin


def finish(results):
    total = 0.0
    for rmap in results:
        total += rmap["partials"].astype(np.float64).sum()
    return np.asarray(0.5 * total / (B * N), dtype=np.float32)


def kernel(pos, f):
    from concourse.bass_utils import run_bass_kernel_spmd

    in_maps, margin = prepare_inputs(pos, f)
    nc = _get_kernel(margin)
    res = run_bass_kernel_spmd(nc, in_maps, list(range(B)))
    return finish(res.results)
